# revision 90
# baseline (speedup 1.0000x reference)
"""CrossScanAttention (bimamba-v3) Trainium2 kernel, v3.

Full inputs -> shard batch across 8 NeuronCores (2 batches/core) -> full output.
Self-contained: hardcodes all shapes; no sibling imports, no file reads.

v3 engine plan (v2 was 278us, DVE 81% busy):
  - PE (6% busy in v2, matmul cost = out-cols only, Ldweights free) absorbs:
    conv (diag-matmuls), B-broadcast (replicated-weight matmuls straight from
    xs, so B rows never exist in SBUF), the 16-state y reduction (identity
    accumulating matmuls into PSUM), D*xs (diag matmul into the same PSUM),
    and the final v-contraction (per-branch accumulating matmuls) - the
    y_sum tensor and all tree adds disappear.
  - Pool takes the dBu multiplies (PSUM B x SBUF u), pool4 folds, gating.
  - dA powers: dA_s = q^a_s with a_s = exp(A_log) = 1..16, so some states
    come from squaring chains (DVE TT, 2x mode) instead of ACT exps
    (LADDER_OPS tunes the ACT/DVE split). dA/dBu tiles are fp16.
  - b-doubling: both local batches ride in one [128, 2, 768] tile for all
    elementwise/ACT work, halving instruction-count overheads.
  - scans stay DVE (walrus rejects Pool TensorScalarPtr), [128, 4*2*770]
    per (branch, state-chunk): 12 scans.
"""

import numpy as np

# ---- problem constants ----
B, C, H, W = 16, 768, 32, 32
D_MODEL, D_INNER, D_STATE, DT_RANK, D_CONV = 64, 128, 16, 4, 4
LN_EPS = 1e-5
NCORES = 8
BL = B // NCORES          # 2 local batches per core
L = C                     # 768
SS = 4                    # states per chunk
TS2 = L + 2               # 768 data + 2 zero break cols

_cached = {}

# p128f column layout
PF_A, PF_CB, PF_DTB, PF_BE, PF_EPS = 0, 48, 51, 54, 58
PF_NCOL = 60

# exponent -> source op. ('exp',) = ACT exp from lnq; ('ts1',) = copy q;
# ('sq', m) = square of exponent-m slot; ('mul', m1, m2) = product.
# sq/mul refs must be <= 1 state-chunk back (dA pool bufs=2).
LADDER_OPS = {2: ('sq', 1), 3: ('mul', 1, 2), 4: ('sq', 2),
              10: ('sq', 5), 11: ('mul', 6, 5), 12: ('sq', 6)}

# which (global chunk idx br*4+sc) h*C multiplies run on DVE vs Pool
HT_DVE = {4, 7, 11}


def _build_nc(use_ladder):
    import concourse.bass as bass
    import concourse.bacc as bacc
    import concourse.tile as tile
    import concourse.mybir as mybir
    from concourse.masks import make_identity
    from contextlib import ExitStack

    f32 = mybir.dt.float32
    bf16 = mybir.dt.bfloat16
    fp16 = mybir.dt.float16
    fp8 = mybir.dt.float8e4
    AL = mybir.AluOpType
    AF = mybir.ActivationFunctionType
    AX = mybir.AxisListType

    ladder = LADDER_OPS if use_ladder else {}

    nc = bacc.Bacc("TRN2", target_bir_lowering=False, debug=False)

    img1 = nc.dram_tensor("img1", (BL, C, H, W), f32, kind="ExternalInput").ap()
    img2 = nc.dram_tensor("img2", (BL, C, H, W), f32, kind="ExternalInput").ap()
    p128f = nc.dram_tensor("p128f", (128, PF_NCOL), f32, kind="ExternalInput").ap()
    p128b = nc.dram_tensor("p128b", (128, 109), bf16, kind="ExternalInput").ap()
    p17f = nc.dram_tensor("p17f", (17, 64), bf16, kind="ExternalInput").ap()
    p4b = nc.dram_tensor("p4b", (4, 384), bf16, kind="ExternalInput").ap()
    p64b = nc.dram_tensor("p64b", (64, 512), bf16, kind="ExternalInput").ap()
    pdiag = nc.dram_tensor("pdiag", (128, 15 * 128), bf16, kind="ExternalInput").ap()
    miscf = nc.dram_tensor("miscf", (1, 2), f32, kind="ExternalInput").ap()
    att_out = nc.dram_tensor("att", (1, BL * L), f32, kind="ExternalOutput").ap()

    def rev(ap):
        """View with the last free dim reversed."""
        steps = [list(x) for x in ap.ap]
        st, n = steps[-1]
        newap = steps[:-1] + [[-st, n]]
        return bass.AP(tensor=ap.tensor, offset=ap.offset + st * (n - 1), ap=newap)

    with nc.allow_low_precision("bf16 intermediate precision is sufficient"), \
         tile.TileContext(nc) as tc, ExitStack() as ctx:
        consts = ctx.enter_context(tc.tile_pool(name="consts", bufs=1))
        imgp = ctx.enter_context(tc.tile_pool(name="imgp", bufs=3))
        ps_tp = ctx.enter_context(tc.tile_pool(name="ps_tp", bufs=1, space="PSUM"))
        ps_mm = ctx.enter_context(tc.tile_pool(name="ps_mm", bufs=2, space="PSUM"))
        ps_yb = ctx.enter_context(tc.tile_pool(name="ps_yb", bufs=1, space="PSUM"))
        stats = ctx.enter_context(tc.tile_pool(name="stats", bufs=26))
        xtp = ctx.enter_context(tc.tile_pool(name="xtp", bufs=4))
        szp = ctx.enter_context(tc.tile_pool(name="szp", bufs=2))
        xpadp = ctx.enter_context(tc.tile_pool(name="xpadp", bufs=3))
        xsp = ctx.enter_context(tc.tile_pool(name="xsp", bufs=3))
        dbcp = ctx.enter_context(tc.tile_pool(name="dbcp", bufs=3))
        qp = ctx.enter_context(tc.tile_pool(name="qp", bufs=3))
        up = ctx.enter_context(tc.tile_pool(name="up", bufs=3))
        dAp = ctx.enter_context(tc.tile_pool(name="dAp", bufs=2))
        dBup = ctx.enter_context(tc.tile_pool(name="dBup", bufs=2))
        hp = ctx.enter_context(tc.tile_pool(name="hp", bufs=2))
        bbcp = ctx.enter_context(tc.tile_pool(name="bbcp", bufs=4))
        cbcp = ctx.enter_context(tc.tile_pool(name="cbcp", bufs=3))
        t2p = ctx.enter_context(tc.tile_pool(name="t2p", bufs=6))
        dramp = ctx.enter_context(tc.tile_pool(name="dramp", bufs=2, space="DRAM"))
        outp = ctx.enter_context(tc.tile_pool(name="outp", bufs=1))

        # ---- constants ----
        c128f = consts.tile([128, PF_NCOL], f32)
        nc.scalar.dma_start(out=c128f, in_=p128f)
        c128b = consts.tile([128, 109], bf16)
        nc.scalar.dma_start(out=c128b, in_=p128b)
        c17 = consts.tile([16, 64], bf16)
        nc.scalar.dma_start(out=c17, in_=p17f[0:16, :])
        c_preb = consts.tile([1, 64], bf16)
        nc.scalar.dma_start(out=c_preb, in_=p17f[16:17, :])
        ones1 = consts.tile([1, 128], bf16)
        nc.vector.memset(ones1[:, :], 1.0)
        c4 = consts.tile([4, 384], bf16)
        nc.scalar.dma_start(out=c4, in_=p4b)
        c64 = consts.tile([64, 512], bf16)
        nc.scalar.dma_start(out=c64, in_=p64b)
        cdiag = consts.tile([128, 15 * 128], bf16)
        nc.scalar.dma_start(out=cdiag, in_=pdiag)
        cmisc = consts.tile([1, 2], f32)
        nc.scalar.dma_start(out=cmisc, in_=miscf)
        identb = consts.tile([128, 128], bf16)
        make_identity(nc, identb[:, :])

        # ---- stages A+B, grouped per image i (i0's B overlaps i1's A) ----
        pooledT = outp.tile([16, 4 * L], bf16)
        imgs = [img1, img2]
        xT = {}

        def stage_ab(i):
            for b in range(BL):
                for k in range(6):
                    if k % 2 == 0:
                        it = imgp.tile([128, 1024], bf16, tag="imgtile")
                    else:
                        it = imgp.tile([128, 1024], f32, tag="imgf32")
                    src = imgs[i][b, k * 128:(k + 1) * 128, :, :].rearrange(
                        "c h w -> c (h w)")
                    if k % 2 == 0:
                        nc.gpsimd.dma_start(out=it, in_=src)
                    else:
                        nc.sync.dma_start(out=it, in_=src)
                    halves = it[:, :].rearrange("p (jr half) -> p jr half",
                                                jr=4, half=256)
                    nc.gpsimd.tensor_tensor(out=halves[:, :, 0:128],
                                            in0=halves[:, :, 0:128],
                                            in1=halves[:, :, 128:256], op=AL.add)
                    h4 = halves[:, :, 0:128].rearrange(
                        "p jr (hb jcwb) -> p jr hb jcwb", hb=4, jcwb=32)
                    nc.gpsimd.tensor_tensor(out=h4[:, :, 0:2, :],
                                            in0=h4[:, :, 0:2, :],
                                            in1=h4[:, :, 2:4, :], op=AL.add)
                    v5f = halves[:, :, 0:64].rearrange(
                        "p jr (hb jc wb) -> p jr jc hb wb", hb=2, jc=4, wb=8)
                    psum = stats.tile([128, 16], bf16, tag="poolsum")
                    nc.vector.tensor_reduce(out=psum, in_=v5f, axis=AX.XY, op=AL.add)
                    pt_ps = ps_tp.tile([64, 128], bf16, tag="tp")
                    nc.tensor.transpose(pt_ps[0:16, :], psum[:, :], identb[:, :])
                    col = (i * BL + b) * L + k * 128
                    nc.vector.tensor_copy(out=pooledT[0:16, col:col + 128],
                                           in_=pt_ps[0:16, :])
            for b in range(BL):
                xt = xtp.tile([64, L], bf16, tag="xT")
                for k in range(6):
                    col = (i * BL + b) * L + k * 128
                    xp_full = ps_mm.tile([128, L], f32, tag="mm", name=f"xq{i}{b}{k}")
                    xp_ps = xp_full[:, 0:64]
                    nc.tensor.matmul(xp_ps, lhsT=pooledT[:, col:col + 128],
                                     rhs=c17[:, :], start=True, stop=False)
                    nc.tensor.matmul(xp_ps, lhsT=ones1[:, :],
                                     rhs=c_preb[:, :], start=False, stop=True)
                    st6 = stats.tile([128, 6], f32, tag="bnst")
                    nc.vector.bn_stats(out=st6, in_=xp_ps)
                    m = stats.tile([128, 2], f32, tag="bnmv", name=f"mv{i}{b}{k}")
                    nc.vector.bn_aggr(out=m, in_=st6)
                    xsb = stats.tile([128, 64], bf16, tag="xpsb", name=f"xp{i}{b}{k}")
                    nc.vector.tensor_copy(out=xsb[:, :], in_=xp_ps)
                    sd = stats.tile([128, 1], f32, tag="lnv", name=f"lnv{i}{b}{k}")
                    nc.scalar.activation(sd[:, :], m[:, 1:2], AF.Sqrt,
                                         bias=c128f[:, PF_EPS:PF_EPS + 1])
                    r = stats.tile([128, 1], f32, tag="rs", name=f"rs{i}{b}{k}")
                    nc.vector.reciprocal(out=r[:, :], in_=sd[:, :])
                    nc.vector.tensor_scalar(out=xsb[:, :], in0=xsb[:, :],
                                            scalar1=m[:, 0:1],
                                            scalar2=r[:, 0:1],
                                            op0=AL.subtract, op1=AL.mult)
                    xn_ps = ps_tp.tile([64, 128], bf16, tag="tp",
                                       name=f"tn{i}{b}{k}")
                    nc.tensor.transpose(xn_ps[:, :], xsb[:, :], identb[:, :])
                    nc.vector.tensor_copy(out=xt[:, k * 128:(k + 1) * 128],
                                          in_=xn_ps[:, :])
                xT[(i, b)] = xt

        # ---- stages C/D1/D2 as per-i / per-br emitters ----
        xpad = {}
        for br in range(3):
            xp = xpadp.tile([128, 2, L + 3], bf16, tag="xpad", name=f"xpad{br}")
            nc.vector.memset(xp[:, :, 0:3], 0.0)
            xpad[br] = xp
        sz = {}
        for i in range(2):
            sz[i] = szp.tile([128, 2, L], bf16, tag="sz", name=f"sz{i}")
        xs_map = {}
        dbc_map = {}
        q_map = {}
        lnq_map = {}
        u_map = {}
        b_dr = {}
        c_dr = {}

        def stage_c(i):
            for b in range(BL):
                for h in range(2):
                    ps = ps_mm.tile([128, L], f32, tag="mm", name=f"ip{i}{b}{h}")
                    wsl = c64[:, i * 256 + h * 128: i * 256 + (h + 1) * 128]
                    nc.tensor.matmul(ps[:, 0:512], lhsT=wsl, rhs=xT[(i, b)][:, 0:512],
                                     start=True, stop=True)
                    nc.tensor.matmul(ps[:, 512:768], lhsT=wsl,
                                     rhs=xT[(i, b)][:, 512:768], start=True, stop=True)
                    be = c128f[:, PF_BE + i * 2 + h: PF_BE + i * 2 + h + 1]
                    if h == 0:
                        brs = [0, 1] if i == 0 else [2]
                        for br in brs:
                            sap = ps[:, :] if br != 1 else rev(ps[:, :])
                            nc.scalar.activation(xpad[br][:, b, 3:L + 3], sap,
                                                 AF.Identity, bias=be)
                    else:
                        nc.scalar.activation(sz[i][:, b, :], ps[:, :], AF.Silu,
                                             bias=be)

        def stage_d1(br):
            xs_map[br] = xsp.tile([128, 2, L], bf16, tag="xs", name=f"xs{br}")
            cv_ps = {}
            for b in range(BL):
                cv = ps_mm.tile([128, L], f32, tag="mm", name=f"cv{br}{b}")
                for k in range(4):
                    dg = cdiag[:, (br * 4 + k) * 128:(br * 4 + k + 1) * 128]
                    nc.tensor.matmul(cv[:, 0:512], lhsT=dg,
                                     rhs=xpad[br][:, b, k:k + 512],
                                     start=(k == 0), stop=False)
                    nc.tensor.matmul(cv[:, 512:768], lhsT=dg,
                                     rhs=xpad[br][:, b, k + 512:k + 768],
                                     start=(k == 0), stop=(k == 3))
                cv_ps[b] = cv
            for b in range(BL):
                nc.scalar.activation(xs_map[br][:, b, :], cv_ps[b][:, :],
                                     AF.Silu, bias=c128f[:, PF_CB + br:PF_CB + br + 1])
            dbc_map[br] = dbcp.tile([36, 2, L], bf16, tag="dbc", name=f"dbc{br}")
            xw = c128b[:, br * 36:(br + 1) * 36]
            for b in range(BL):
                dfull = ps_mm.tile([128, L], f32, tag="mm", name=f"db{br}{b}")
                dps = dfull[0:36, :]
                nc.tensor.matmul(dps[:, 0:512], lhsT=xw, rhs=xs_map[br][:, b, 0:512],
                                 start=True, stop=True)
                nc.tensor.matmul(dps[:, 512:768], lhsT=xw,
                                 rhs=xs_map[br][:, b, 512:768], start=True, stop=True)
                nc.vector.tensor_copy(out=dbc_map[br][:, b, :], in_=dps[:, :])

        def stage_d2(br):
            q_map[br] = qp.tile([128, 2, L], fp16, tag="q", name=f"q{br}")
            dw = c4[:, br * 128:(br + 1) * 128]
            for b in range(BL):
                dtps = ps_mm.tile([128, L], f32, tag="mm", name=f"dt{br}{b}")
                nc.tensor.matmul(dtps[:, 0:512], lhsT=dw, rhs=dbc_map[br][0:4, b, 0:512],
                                 start=True, stop=True)
                nc.tensor.matmul(dtps[:, 512:768], lhsT=dw,
                                 rhs=dbc_map[br][0:4, b, 512:768], start=True, stop=True)
                nc.scalar.activation(q_map[br][:, b, :], dtps[:, :],
                                     AF.Sigmoid, scale=-1.0,
                                     bias=c128f[:, PF_DTB + br:PF_DTB + br + 1])
            bd = dramp.tile([16, 2 * L], fp8, tag="bdr", name=f"bdr{br}")
            nc.gpsimd.dma_start(out=bd[:, :],
                              in_=dbc_map[br][4:20, :, :].rearrange("s b t -> s (b t)"))
            b_dr[br] = bd
            cd = dramp.tile([16, 2 * L], bf16, tag="cdr", name=f"cdr{br}")
            nc.sync.dma_start(out=cd[:, :],
                              in_=dbc_map[br][20:36, :, :].rearrange("s b t -> s (b t)"))
            c_dr[br] = cd
            nc.scalar.activation(q_map[br][:, :, :], q_map[br][:, :, :], AF.Ln)
            lnq_map[br] = q_map[br]
            u = up.tile([128, 2, L], bf16, tag="u", name=f"u{br}")
            nc.gpsimd.tensor_tensor(out=u[:, :, :], in0=lnq_map[br][:, :, :],
                                    in1=xs_map[br][:, :, :], op=AL.mult)
            u_map[br] = u

        # ---- stage D3: per (br, sc): dA, dBu, scan, h*C, PE y-accumulation ----
        yb_ps = {}
        t2_map = {}

        def stage_d3(br):
            dA_t = {}
            yb_t = ps_yb.tile([128, 2, L], f32, tag="yb", name=f"yb{br}")
            for b in range(BL):
                yb_ps[(br, b)] = yb_t[:, b, :]
            for sc in range(4):
                dA = dAp.tile([128, SS, 2, TS2], fp16, tag="dA",
                              name=f"dA{br}{sc}")
                dA_t[sc] = dA
                nc.vector.memset(dA[:, :, :, L:TS2], 0.0)
                lnq = lnq_map[br]
                for sl in range(SS):
                    n = sc * SS + sl + 1  # exponent = state index + 1
                    slot = dA[:, sl, :, 0:L]
                    op = ladder.get(n, ('exp',))
                    def sv(m):
                        msc, msl = divmod(m - 1, SS)
                        return dA_t[msc][:, msl, :, 0:L]
                    if op[0] == 'exp':
                        si = br * 16 + n - 1
                        nc.scalar.activation(slot, lnq[:, :, :], AF.Exp,
                                             scale=c128f[:, PF_A + si:PF_A + si + 1])
                    elif op[0] == 'sqa':
                        nc.scalar.activation(slot, sv(op[1]), AF.Square)
                    else:
                        in0 = sv(op[1])
                        in1 = in0 if op[0] == 'sq' else sv(op[2])
                        eng = nc.vector if op[0] == 'sq' else nc.gpsimd
                        eng.tensor_tensor(out=slot, in0=in0, in1=in1,
                                          op=AL.mult)

                dBu = dBup.tile([128, SS, 2, TS2], fp16, tag="dBu")
                nc.vector.memset(dBu[:, :, :, L:TS2], 0.0)
                uap = u_map[br][:, :, :]
                u_bc = bass.AP(tensor=uap.tensor, offset=uap.offset,
                               ap=[list(uap.ap[0]), [0, 2], list(uap.ap[1]),
                                   list(uap.ap[2])])
                for hf in range(2):
                    bbc = bbcp.tile([128, 2, 2, L], fp8, tag="bb")
                    bsl = b_dr[br][sc * SS + hf * 2: sc * SS + hf * 2 + 2, :]
                    bsrc = bass.AP(tensor=bsl.tensor, offset=bsl.offset,
                                   ap=[[0, 128]] + [list(x) for x in bsl.ap])
                    nc.sync.dma_start(
                        out=bbc[:, :, :, :].rearrange("p s b t -> p s (b t)"),
                        in_=bsrc)
                    nc.gpsimd.tensor_tensor(out=dBu[:, hf * 2:hf * 2 + 2, :, 0:L],
                                            in0=u_bc, in1=bbc[:, :, :, :],
                                            op=AL.mult)

                cbc_h = []
                for hf in range(2):
                    cbc = cbcp.tile([128, 2, 2, L], bf16, tag="cb")
                    csl = c_dr[br][sc * SS + hf * 2: sc * SS + hf * 2 + 2, :]
                    src_ap = bass.AP(tensor=csl.tensor, offset=csl.offset,
                                     ap=[[0, 128]] + [list(x) for x in csl.ap])
                    nc.sync.dma_start(
                        out=cbc[:, :, :, :].rearrange("p s b t -> p s (b t)"),
                        in_=src_ap)
                    cbc_h.append(cbc)

                h = hp.tile([128, SS, 2, TS2], bf16, tag="h")
                for lo, hi in ((0, 2), (2, 4)):
                    sl2 = slice(lo, hi)
                    nc.vector.tensor_tensor_scan(
                        out=h[:, sl2, :, :].rearrange("p s b t -> p (s b t)"),
                        data0=dA[:, sl2, :, :].rearrange("p s b t -> p (s b t)"),
                        data1=dBu[:, sl2, :, :].rearrange("p s b t -> p (s b t)"),
                        initial=0.0, op0=AL.mult, op1=AL.add)

                heng = nc.vector if (br * 4 + sc) in HT_DVE else nc.gpsimd
                for hf in range(2):
                    heng.tensor_tensor(out=h[:, hf * 2:hf * 2 + 2, :, 0:L],
                                       in0=h[:, hf * 2:hf * 2 + 2, :, 0:L],
                                       in1=cbc_h[hf][:, :, :, :], op=AL.mult)

                for b in range(BL):
                    for sl in range(SS):
                        first = (sc == 0 and sl == 0)
                        nc.tensor.matmul(yb_ps[(br, b)][:, 0:512],
                                         lhsT=identb[:, :], rhs=h[:, sl, b, 0:512],
                                         start=first, stop=False)
                        nc.tensor.matmul(yb_ps[(br, b)][:, 512:768],
                                         lhsT=identb[:, :],
                                         rhs=h[:, sl, b, 512:768],
                                         start=first, stop=False)

            # D*xs into the same accumulator (stop), then gating -> t2
            for b in range(BL):
                dg = cdiag[:, (12 + br) * 128:(13 + br) * 128]
                nc.tensor.matmul(yb_ps[(br, b)][:, 0:512], lhsT=dg,
                                 rhs=xs_map[br][:, b, 0:512],
                                 start=False, stop=False)
                nc.tensor.matmul(yb_ps[(br, b)][:, 512:768], lhsT=dg,
                                 rhs=xs_map[br][:, b, 512:768],
                                 start=False, stop=True)
                t2 = t2p.tile([128, L], bf16, tag="t2", name=f"t2{br}{b}")
                img_i = 0 if br < 2 else 1
                yb_in = yb_ps[(br, b)]
                if br == 1:
                    yb_in = rev(yb_in)
                nc.vector.tensor_tensor(out=t2[:, :], in0=yb_in,
                                        in1=sz[img_i][:, b, :], op=AL.mult)
                t2_map[(br, b)] = t2

        stage_ab(0)
        stage_c(0)
        stage_d1(0)
        stage_d2(0)
        stage_d1(1)
        stage_d2(1)
        stage_ab(1)
        stage_c(1)
        stage_d1(2)
        stage_d2(2)
        stage_d3(0)
        stage_d3(1)
        stage_d3(2)

        # ---- final head: per-b accumulating v-contraction + sigmoid ----
        vcol = c128b[:, 108:109]
        for b in range(BL):
            lgf = ps_mm.tile([128, L], f32, tag="mm", name=f"lg{b}")
            lg = lgf[0:1, :]
            for br in range(3):
                nc.tensor.matmul(lg[:, 0:512], lhsT=vcol, rhs=t2_map[(br, b)][:, 0:512],
                                 start=(br == 0), stop=False)
                nc.tensor.matmul(lg[:, 512:768], lhsT=vcol,
                                 rhs=t2_map[(br, b)][:, 512:768],
                                 start=(br == 0), stop=(br == 2))
            att_b = outp.tile([1, L], bf16, tag="attb", name=f"att{b}")
            nc.scalar.activation(att_b[:, :], lg[:, :], AF.Sigmoid,
                                 scale=0.5, bias=cmisc[0:1, 0:1])
            nc.vector.tensor_scalar_add(att_b[:, :], att_b[:, :], 1e-6)
            nc.gpsimd.dma_start(out=att_out[:, b * L:(b + 1) * L], in_=att_b[:, :])

    nc.compile()
    return nc


def _pack_params(inputs):
    import ml_dtypes
    gi = lambda k: np.asarray(inputs[k], dtype=np.float32)
    tags = ("f", "b", "s")

    a_vals = np.stack([np.exp(gi("A_log_" + t)) for t in tags])  # (3, 128, 16)
    use_ladder = True
    for n, op in LADDER_OPS.items():
        col = a_vals[:, :, n - 1]
        if not np.allclose(col, float(n), atol=1e-3):
            use_ladder = False
        if op[0] == 'sq' and not np.allclose(a_vals[:, :, op[1] - 1] * 2.0,
                                             col, atol=1e-3):
            use_ladder = False
        if op[0] == 'mul' and not np.allclose(
                a_vals[:, :, op[1] - 1] + a_vals[:, :, op[2] - 1], col, atol=1e-3):
            use_ladder = False

    p128f = np.zeros((128, PF_NCOL), np.float32)
    p128b = np.zeros((128, 109), np.float32)
    pdiag = np.zeros((128, 15 * 128), np.float32)
    for t, tag in enumerate(tags):
        p128f[:, PF_A + t * 16: PF_A + 16 + t * 16] = a_vals[t]
        p128f[:, PF_CB + t] = gi("conv_b_" + tag)
        p128f[:, PF_DTB + t] = -gi("dtproj_b_" + tag)   # sigmoid(-x - b)
        xw = gi("xproj_w_" + tag).T.copy()              # (128, 36) in: dt|B|C
        p128b[:, t * 36:t * 36 + 4] = xw[:, 0:DT_RANK]
        # sign-flip B folds u = lnq*xs = -dt*xs
        p128b[:, t * 36 + 4:t * 36 + 20] = -xw[:, DT_RANK:DT_RANK + D_STATE]
        p128b[:, t * 36 + 20:(t + 1) * 36] = xw[:, DT_RANK + D_STATE:]
        cw = gi("conv_w_" + tag)                        # (128, 4)
        for k in range(4):
            blk = (t * 4 + k) * 128
            pdiag[:, blk:blk + 128] = np.diag(cw[:, k])
        pdiag[:, (12 + t) * 128:(13 + t) * 128] = np.diag(gi("D_" + tag))
    p128b[:, 108] = gi("out_proj_w").T @ gi("post_w")[0]
    p128f[:, PF_EPS] = LN_EPS
    ln_g, ln_b = gi("ln_g"), gi("ln_b")
    w1t = gi("in_proj_w").T
    w2t = gi("in_proj_s_w").T
    b1 = ln_b @ w1t
    b2 = ln_b @ w2t
    p128f[:, PF_BE + 0] = b1[0:128]
    p128f[:, PF_BE + 1] = b1[128:256]
    p128f[:, PF_BE + 2] = b2[0:128]
    p128f[:, PF_BE + 3] = b2[128:256]

    p17f = np.zeros((17, 64), np.float32)
    p17f[0:16] = gi("pre_w").T / 64.0
    p17f[16] = gi("pre_b")

    p4b = np.zeros((4, 384), np.float32)
    for t, tag in enumerate(tags):
        p4b[:, t * 128:(t + 1) * 128] = gi("dtproj_w_" + tag).T

    p64b = np.zeros((64, 512), np.float32)
    p64b[:, 0:256] = w1t * ln_g[:, None]
    p64b[:, 256:512] = w2t * ln_g[:, None]

    miscf = np.zeros((1, 2), np.float32)
    miscf[0, 0] = 0.5 * float(gi("post_b").reshape(-1)[0])

    bf = ml_dtypes.bfloat16
    return {
        "p128f": p128f,
        "p128b": p128b.astype(bf),
        "p17f": p17f.astype(bf),
        "p4b": p4b.astype(bf),
        "p64b": p64b.astype(bf),
        "pdiag": pdiag.astype(bf),
        "miscf": miscf,
    }, use_ladder


def get_nc(use_ladder=True):
    key = ("nc", use_ladder)
    if key not in _cached:
        _cached[key] = _build_nc(use_ladder)
    return _cached[key]


def make_in_maps(inputs):
    params, use_ladder = _pack_params(inputs)
    img1 = np.ascontiguousarray(np.asarray(inputs["img1_features"], np.float32))
    img2 = np.ascontiguousarray(np.asarray(inputs["img2_features"], np.float32))
    in_maps = []
    for c in range(NCORES):
        m = dict(params)
        m["img1"] = np.ascontiguousarray(img1[c * BL:(c + 1) * BL])
        m["img2"] = np.ascontiguousarray(img2[c * BL:(c + 1) * BL])
        in_maps.append(m)
    return in_maps, use_ladder


def kernel(**inputs):
    from concourse.bass_utils import run_bass_kernel_spmd

    in_maps, use_ladder = make_in_maps(inputs)
    nc = get_nc(use_ladder)
    res = run_bass_kernel_spmd(nc, in_maps, core_ids=list(range(NCORES)))
    outs = [r["att"].reshape(BL, L) for r in res.results]
    att = np.concatenate(outs, axis=0)
    return att.reshape(B, C, 1, 1).astype(np.float32)
